# revision 50
# baseline (speedup 1.0000x reference)
"""Distributed Trainium2 attention kernel (8 NeuronCores).

Sharding: 4-way data parallel over batch x 2-way tensor parallel over heads.
Core c handles batch c//2 and head-group c%2 (8 of 16 heads).
Each core computes q/k/v projections for its head group, rotary+rms-norm,
full non-causal attention for its 8 heads, and a partial output projection
(row-parallel). The host sums the two partials per batch (the unshard step
for row-parallel linear) -- no device collective needed.

Compute dtype: bf16 matmuls (fp32 matmul is 4x slower on PE), fp32 PSUM
accumulation, rotary/rms/softmax math in fp32.

Structure per core:
  Phase 1 (per 128-row t-tile): QKV projection matmuls; rotary+rms for q
  and k merged into single [128,1024] DVE ops (rms stats taken from
  PRE-rotary values -- rotation preserves the per-head norm -- so they
  never serialize with the rotary chain); PE-transposes of q/k deferred
  one tile so the PE FIFO always has dense QKV work; v staged per-head
  with a fused ones-column (softmax denominator).
  Phase 2: per (512-wide q-chunk, head-PAIR, s-tile): the pair's two score
  matmuls are issued adjacently on PE row-groups (0,0)/(64,0) so they run
  CONCURRENTLY (row-tiled packing; also keeps the HAM clock gate fed with
  full-width activity -> 2.4 GHz). One wide exp on ScalarE over the pair's
  [128,1024] psum tile; attn@V per head with V stationary (ones column
  rides as psum row 64 = softmax denominator). Steady state is ACT(exp)-
  bound at ~1.09us/iter with the Scalar engine ~96% busy.
  PSUM: 2x psc[128,1024] ping-pong (4 banks) + 4x ya[65,512] (4 banks).
  ya is 4-deep because a matmul that WARs a recently-read psum buffer
  inherits, via DVE sem-increment coalescing, a wait on everything behind
  that read in the DVE FIFO -- with 4 buffers the WAR lands 2 pairs back.
  Normalize reads ya psum directly (no evacuation copies): denominator
  rows -> partition 0, gpsimd broadcast, reciprocal_approx_fast, two DVE
  multiplies writing bf16 yT. Out-projection runs as a tail reusing the
  psc pool banks (interleaving it into the attention loop needs a free
  psum bank, and all 8 are committed).
"""
import sys
import os
from contextlib import ExitStack

if '/opt/trn_rl_repo' not in sys.path:
    sys.path.insert(0, '/opt/trn_rl_repo')

import numpy as np
import ml_dtypes

bf16 = ml_dtypes.bfloat16

T = 4096
D = 1024
HL = 8          # local heads per core
HD = 64
NT = T // 128   # 32 t-tiles
KT = D // 128   # 8 contraction tiles for projections
NCH = 8         # chunks of 512 along t for attention
CW = 512        # chunk width (query columns per head per iteration)
PAIRS = 4       # head pairs per core
EPS = 1.1920928955078125e-07


def build():
    from concourse import bacc, tile, mybir

    BF16 = mybir.dt.bfloat16
    F32 = mybir.dt.float32
    AF = mybir.ActivationFunctionType
    ALU = mybir.AluOpType
    AX = mybir.AxisListType

    nc = bacc.Bacc()
    # x pre-arranged host-side to [partition, tile*KT*128] so every per-tile
    # load is one contiguous 2KB-per-partition slab (the old transposed
    # gather burned ~90us per DMA queue in descriptors and gated startup)
    xP = nc.declare_dram_parameter("xP", [128, NT * KT * 128], BF16,
                                   isOutput=False)
    wqT = nc.declare_dram_parameter("wqT", [D, 512], BF16, isOutput=False)
    wkT = nc.declare_dram_parameter("wkT", [D, 512], BF16, isOutput=False)
    wvT = nc.declare_dram_parameter("wvT", [D, 512], BF16, isOutput=False)
    woT = nc.declare_dram_parameter("woT", [512, D], BF16, isOutput=False)
    cos2 = nc.declare_dram_parameter("cos2", [128, NT * 64], BF16,
                                     isOutput=False)
    ss = nc.declare_dram_parameter("ss", [128, NT * 32], BF16,
                                   isOutput=False)
    ident = nc.declare_dram_parameter("ident", [128, 128], BF16, isOutput=False)
    out = nc.declare_dram_parameter("out", [T, D], F32, isOutput=True)

    with tile.TileContext(nc) as tc:
        with tc.tile_pool(name="persist", bufs=1) as persist:
            # ---- persistent stores (live across both phases) ----
            # qT split per 1024-chunk so phase 2 chunk c only depends on its
            # own 8 q t-tiles; kT/vaug are needed in full by every chunk.
            qTc = [persist.tile([128, PAIRS, 1024], BF16, tag=f"qT{c}",
                                name=f"qT{c}") for c in range(4)]
            kT = persist.tile([128, PAIRS, T], BF16, tag="kT")
            vaug = persist.tile([128, NT, HL, 65], BF16, tag="vaug")
            wo_sb = persist.tile([128, 4, D], BF16, tag="wo_sb")
            id_sb = persist.tile([128, 128], BF16, tag="id_sb")
            eps_t = persist.tile([128, 1], F32, tag="eps_t")

            nc.vector.memset(vaug[:, :, :, 64:65], 1.0)
            nc.vector.memset(eps_t[:], EPS)

            phase1 = ExitStack()
            with phase1:
                wpool = phase1.enter_context(tc.tile_pool(name="wpool", bufs=1))
                xcolp = phase1.enter_context(tc.tile_pool(name="xcolp", bufs=3))
                scr = phase1.enter_context(tc.tile_pool(name="scr", bufs=3))
                small = phase1.enter_context(tc.tile_pool(name="small", bufs=4))
                ps_qkv = phase1.enter_context(
                    tc.tile_pool(name="ps_qkv", bufs=4, space="PSUM"))
                ps_tr = phase1.enter_context(
                    tc.tile_pool(name="ps_tr", bufs=4, space="PSUM"))

                def load_xcol(t):
                    xc = xcolp.tile([128, KT, 128], BF16, tag="xcol")
                    nc.sync.dma_start(
                        xc[:],
                        xP[:, t * 1024:(t + 1) * 1024].rearrange(
                            "p (k c) -> p k c", k=KT))
                    return xc

                # interleave the first x tiles with the weight loads: the
                # first QKV matmul needs xcol(0)+w_q, and each dma_start
                # costs ~600ns of sequencer issue time, so issue order
                # directly sets how soon the PE can start
                xcols = {0: load_xcol(0)}
                w_sb = {}
                cos_sb = wpool.tile([128, NT, 64], BF16, tag="cos_sb")
                ss_sb = wpool.tile([128, NT, 32], BF16, tag="ss_sb")
                for pf, (name, param) in zip(
                        (1, 2, None),
                        (("q", wqT), ("k", wkT), ("v", wvT))):
                    w_sb[name] = wpool.tile(
                        [128, KT, 512], BF16, tag=f"w{name}",
                        name=f"w_{name}_sb")
                    for ki in range(KT):
                        nc.sync.dma_start(
                            w_sb[name][:, ki, :],
                            param[ki * 128:(ki + 1) * 128, :])
                    if name == "q":
                        # cos/sin feed the first rotary (~16us in) -- issue
                        # before the k/v weights so they arrive in time
                        nc.sync.dma_start(
                            cos_sb[:],
                            cos2[:].rearrange("p (t d) -> p t d", t=NT))
                        nc.sync.dma_start(
                            ss_sb[:], ss[:].rearrange("p (t d) -> p t d",
                                                      t=NT))
                        nc.sync.dma_start(id_sb[:], ident[:])
                    if pf is not None:
                        xcols[pf] = load_xcol(pf)
                nc.sync.dma_start(
                    wo_sb[:], woT[:].rearrange("(k p) n -> p k n", p=128))

                # ============ Phase 1: QKV + rotary + rms + transpose =======
                # The q/k transposes of tile t are deferred until after tile
                # t+1's QKV matmuls are issued: the PE FIFO then always has
                # dense work while the rotary chain (DVE) catches up, instead
                # of stalling on the transposes' qn dependency every tile.
                pending_tr = []
                for t in range(NT):
                    xcol = xcols.pop(t) if t in xcols else load_xcol(t)

                    ps = {}
                    for name in ("q", "k", "v"):
                        ps[name] = ps_qkv.tile([128, 512], F32, tag="pqkv",
                                               name=f"ps_{name}")
                        for ki in range(KT):
                            nc.tensor.matmul(
                                ps[name][:], xcol[:, ki, :],
                                w_sb[name][:, ki, :],
                                start=(ki == 0), stop=(ki == KT - 1))

                    # flush previous tile's transposes (PE busy with QKV now)
                    for qn_, store_, tcol_ in pending_tr:
                        for cb in range(4):
                            tp = ps_tr.tile([128, 128], BF16, tag="tp")
                            nc.tensor.transpose(
                                tp[:], qn_[:, cb * 128:(cb + 1) * 128],
                                id_sb[:])
                            nc.scalar.copy(
                                store_[:, cb, tcol_:tcol_ + 128], tp[:])
                    pending_tr = []

                    # v -> vaug rows 0:64 per head (ScalarE copy, bf16 cast)
                    nc.scalar.copy(
                        vaug[:, t, :, 0:64],
                        ps["v"][:].rearrange("p (h d) -> p h d", h=HL))

                    # rotary + rms for q and k, processed as ONE [128, 1024]
                    # block (q cols 0:512, k cols 512:1024) to halve the DVE
                    # per-op overhead.  rms stats come from PRE-rotary values
                    # (rotation preserves the per-head norm).
                    ctb = cos_sb[:, t, :].unsqueeze(1).unsqueeze(1)\
                        .broadcast_to([128, 2, HL, 64])
                    sb = ss_sb[:, t, :].unsqueeze(1).unsqueeze(1)\
                        .broadcast_to([128, 2, HL, 32])
                    qb = scr.tile([128, 1024], BF16, tag="qb", name="qb_qk")
                    nc.scalar.copy(qb[:, 0:512], ps["q"][:])
                    nc.scalar.copy(qb[:, 512:1024], ps["k"][:])
                    sq = scr.tile([128, 1024], BF16, tag="sq", name="sq_qk")
                    nc.vector.tensor_tensor(sq[:], qb[:], qb[:], op=ALU.mult)
                    ms8 = small.tile([128, 2 * HL], F32, tag="ms8",
                                     name="ms8_qk")
                    nc.vector.tensor_reduce(
                        ms8[:], sq[:].rearrange("p (n h d) -> p (n h) d",
                                                n=2, h=HL),
                        axis=AX.X, op=ALU.add)
                    rms = small.tile([128, 2 * HL], F32, tag="rms",
                                     name="rms_qk")
                    nc.scalar.activation(
                        rms[:], ms8[:], AF.Sqrt, scale=1.0 / HD,
                        bias=eps_t[:])
                    rinv = small.tile([128, 2 * HL], F32, tag="rinv",
                                      name="rinv_qk")
                    nc.vector.reciprocal(rinv[:], rms[:])
                    t1 = scr.tile([128, 1024], BF16, tag="t1", name="t1_qk")
                    nc.vector.tensor_tensor(
                        t1[:].rearrange("p (n h d) -> p n h d", n=2, h=HL),
                        qb[:].rearrange("p (n h d) -> p n h d", n=2, h=HL),
                        ctb, op=ALU.mult)
                    qb4 = qb[:].rearrange("p (n h u d) -> p n h u d",
                                          n=2, h=HL, u=2)
                    r = scr.tile([128, 1024], BF16, tag="r", name="r_qk")
                    r4 = r[:].rearrange("p (n h u d) -> p n h u d",
                                        n=2, h=HL, u=2)
                    t14 = t1[:].rearrange("p (n h u d) -> p n h u d",
                                          n=2, h=HL, u=2)
                    nc.vector.tensor_tensor(
                        r4[:, :, :, 0, :], qb4[:, :, :, 1, :], sb,
                        op=ALU.mult)
                    nc.vector.tensor_tensor(
                        r4[:, :, :, 1, :], qb4[:, :, :, 0, :], sb,
                        op=ALU.mult)
                    nc.vector.tensor_tensor(
                        r4[:, :, :, 0, :], t14[:, :, :, 0, :],
                        r4[:, :, :, 0, :], op=ALU.add)
                    nc.vector.tensor_tensor(
                        r4[:, :, :, 1, :], t14[:, :, :, 1, :],
                        r4[:, :, :, 1, :], op=ALU.subtract)
                    qn = scr.tile([128, 1024], BF16, tag="qn", bufs=3,
                                  name="qn_qk")
                    nc.vector.tensor_tensor(
                        qn[:].rearrange("p (m d) -> p m d", m=2 * HL),
                        r[:].rearrange("p (m d) -> p m d", m=2 * HL),
                        rinv[:].unsqueeze(2).broadcast_to([128, 2 * HL, 64]),
                        op=ALU.mult)
                    pending_tr.append((qn[:, 0:512], qTc[t // 8],
                                       (t % 8) * 128))
                    pending_tr.append((qn[:, 512:1024], kT, t * 128))

                # flush the last tile's transposes
                for qn_, store_, tcol_ in pending_tr:
                    for cb in range(4):
                        tp = ps_tr.tile([128, 128], BF16, tag="tp")
                        nc.tensor.transpose(
                            tp[:], qn_[:, cb * 128:(cb + 1) * 128],
                            id_sb[:])
                        nc.scalar.copy(
                            store_[:, cb, tcol_:tcol_ + 128], tp[:])

            # ============ Phase 2: attention ================================
            with (
                tc.tile_pool(name="p2persist", bufs=1) as p2persist,
                tc.tile_pool(name="zp2", bufs=4) as zp2,
                tc.tile_pool(name="nrm", bufs=2) as nrm,
                tc.tile_pool(name="ostg2", bufs=4) as ostg2,
                tc.tile_pool(name="ps_sc", bufs=2, space="PSUM") as ps_sc,
                tc.tile_pool(name="ps_ya", bufs=4, space="PSUM") as ps_ya,
            ):
                yTn = [p2persist.tile([128, PAIRS, CW], BF16, tag=f"yT{c}",
                                      name=f"yT{c}") for c in range(NCH)]

                # PE warm-up: dense FULL-128-row back-to-back matmuls tied to
                # kT so they run right at phase-2 entry. Re-arms the HAM
                # clock gate before the steady loop (64-row bursts do NOT
                # re-arm it -- learned from the previous kernel's trace).
                wup = ps_sc.tile([128, 2 * CW], F32, tag="psc", name="wup")
                for i in range(16):
                    nc.tensor.matmul(
                        wup[:, 0:512], kT[:, 0, 0:128], kT[:, 0, 0:512],
                        start=(i == 0), stop=(i == 15))

                for ch in range(NCH):
                    qt = qTc[ch // 2]
                    qof = (ch % 2) * 512
                    for pr in range(PAIRS):
                        # bufs=4: each pair's accumulators WAR a buffer last
                        # read TWO pairs back (~70us slack) -- a recent-WAR
                        # here stalls the PE FIFO for the whole normalize
                        # chain (sem-increment coalescing on the DVE stream).
                        ya_e = ps_ya.tile([65, CW], F32, tag="pya",
                                          name="ya_e")
                        ya_o = ps_ya.tile([65, CW], F32, tag="pya",
                                          name="ya_o")
                        for s in range(NT):
                            ssl = slice(s * 128, (s + 1) * 128)
                            psc = ps_sc.tile([128, 2 * CW], F32, tag="psc")
                            # pair's score matmuls: adjacent issue, distinct
                            # PE row-groups -> concurrent execution
                            nc.tensor.matmul(
                                psc[:, 0:CW], kT[0:64, pr, ssl],
                                qt[0:64, pr, qof:qof + CW],
                                start=True, stop=True,
                                tile_position=(0, 0))
                            nc.tensor.matmul(
                                psc[:, CW:2 * CW], kT[64:128, pr, ssl],
                                qt[64:128, pr, qof:qof + CW],
                                start=True, stop=True,
                                tile_position=(64, 0))
                            z = zp2.tile([128, 2 * CW], BF16, tag="z")
                            nc.scalar.activation(
                                z[:], psc[:], AF.Exp, scale=0.125)
                            nc.tensor.matmul(
                                ya_e[:], vaug[:, s, 2 * pr, :], z[:, 0:CW],
                                start=(s == 0), stop=(s == NT - 1))
                            nc.tensor.matmul(
                                ya_o[:], vaug[:, s, 2 * pr + 1, :],
                                z[:, CW:2 * CW],
                                start=(s == 0), stop=(s == NT - 1))
                        # normalize straight out of psum: denominator rows ->
                        # partition 0, broadcast (gpsimd), approx-reciprocal,
                        # then the multiplies read ya PSUM directly and write
                        # normalized bf16 yT (no intermediate evacuation).
                        dtmp = nrm.tile([1, 2 * CW], F32, tag="dtmp", bufs=2)
                        nc.vector.tensor_copy(dtmp[:, 0:CW], ya_e[64:65, :])
                        nc.vector.tensor_copy(dtmp[:, CW:2 * CW],
                                              ya_o[64:65, :])
                        bc = nrm.tile([64, 2 * CW], F32, tag="bc", bufs=2)
                        nc.gpsimd.partition_broadcast(bc[:], dtmp[:],
                                                      channels=64)
                        bcr = nrm.tile([64, 2 * CW], F32, tag="bcr", bufs=2)
                        nc.vector.reciprocal_approx_fast(bcr[:], bc[:])
                        nc.vector.tensor_tensor(
                            yTn[ch][0:64, pr, :], ya_e[0:64, :],
                            bcr[:, 0:CW], op=ALU.mult)
                        nc.vector.tensor_tensor(
                            yTn[ch][64:128, pr, :], ya_o[0:64, :],
                            bcr[:, CW:2 * CW], op=ALU.mult)

                # out-projection tail (PSUM banks freed by the attention
                # loop; po tiles reuse the ps_sc pool)
                for ch in range(NCH):
                    for tt in range(4):
                        po = ps_sc.tile([128, 2 * CW], F32, tag="psc",
                                        name="po")
                        for oc in range(2):
                            for kp in range(4):
                                nc.tensor.matmul(
                                    po[:, oc * 512:(oc + 1) * 512],
                                    yTn[ch][:, kp, tt * 128:(tt + 1) * 128],
                                    wo_sb[:, kp, oc * 512:(oc + 1) * 512],
                                    start=(kp == 0), stop=(kp == 3))
                        ost = ostg2.tile([128, 2 * CW], F32, tag="ost")
                        nc.vector.tensor_copy(ost[:], po[:])
                        tsl = slice(ch * CW + tt * 128,
                                    ch * CW + (tt + 1) * 128)
                        nc.sync.dma_start(out[tsl, :], ost[:])

    nc.compile()
    return nc


_CACHE = {}


def _get_nc():
    if "nc" not in _CACHE:
        _CACHE["nc"] = build()
    return _CACHE["nc"]


def _prep_inputs(x, cos, sin, wq, wk, wv, wo):
    x = np.asarray(x, dtype=np.float32)
    cos = np.asarray(cos, dtype=np.float32).reshape(T, 32)
    sin = np.asarray(sin, dtype=np.float32).reshape(T, 32)
    wq = np.asarray(wq, dtype=np.float32)
    wk = np.asarray(wk, dtype=np.float32)
    wv = np.asarray(wv, dtype=np.float32)
    wo = np.asarray(wo, dtype=np.float32)

    # pre-arrange to the exact SBUF layouts (contiguous per-partition DMAs)
    cos2 = np.ascontiguousarray(
        np.concatenate([cos, cos], axis=1).reshape(32, 128, 64)
        .transpose(1, 0, 2).reshape(128, -1))
    ss = np.ascontiguousarray(
        sin.reshape(32, 128, 32).transpose(1, 0, 2).reshape(128, -1))
    ident = np.eye(128, dtype=bf16)

    in_maps = []
    for c in range(8):
        b, hg = c // 2, c % 2
        rows = slice(hg * 512, (hg + 1) * 512)
        xp = np.ascontiguousarray(
            x[b].T.reshape(8, 128, 32, 128).transpose(1, 2, 0, 3)
            .reshape(128, -1))

        in_maps.append({
            "xP": xp.astype(bf16),
            "wqT": np.ascontiguousarray(wq[rows, :].T).astype(bf16),
            "wkT": np.ascontiguousarray(wk[rows, :].T).astype(bf16),
            "wvT": np.ascontiguousarray(wv[rows, :].T).astype(bf16),
            "woT": np.ascontiguousarray(wo[:, rows].T).astype(bf16),
            "cos2": cos2.astype(bf16),
            "ss": ss.astype(bf16),
            "ident": ident,
        })
    return in_maps


def _run(in_maps, trace=False):
    from concourse.bass_utils import run_bass_kernel_spmd

    nc = _get_nc()
    res = run_bass_kernel_spmd(nc, in_maps, core_ids=list(range(8)),
                               trace=trace)
    parts = [res.results[c]["out"] for c in range(8)]
    full = np.stack([parts[2 * b] + parts[2 * b + 1] for b in range(4)])
    return full.astype(np.float32), res


def kernel(x, cos, sin, wq, wk, wv, wo):
    in_maps = _prep_inputs(x, cos, sin, wq, wk, wv, wo)
    full, _ = _run(in_maps, trace=False)
    return full


# revision 51
# speedup vs baseline: 1.1976x; 1.1976x over previous
"""Distributed Trainium2 attention kernel (8 NeuronCores).

Sharding: 4-way data parallel over batch x 2-way tensor parallel over heads.
Core c handles batch c//2 and head-group c%2 (8 of 16 heads).
Each core computes q/k/v projections for its head group, rotary+rms-norm,
full non-causal attention for its 8 heads, and a partial output projection
(row-parallel). The host sums the two partials per batch (the unshard step
for row-parallel linear) -- no device collective needed.

Compute dtype: bf16 matmuls (fp32 matmul is 4x slower on PE), fp32 PSUM
accumulation, rotary/rms/softmax math in fp32.

Structure per core:
  Phase 1 (per 128-row t-tile): QKV projection matmuls; rotary+rms for q
  and k merged into single [128,1024] DVE ops (rms stats taken from
  PRE-rotary values -- rotation preserves the per-head norm -- so they
  never serialize with the rotary chain); PE-transposes of q/k deferred
  one tile so the PE FIFO always has dense QKV work; v staged per-head
  with a fused ones-column (softmax denominator).
  Phase 2: per (512-wide q-chunk, head-PAIR, s-tile): the pair's two score
  matmuls are issued adjacently on PE row-groups (0,0)/(64,0) so they run
  CONCURRENTLY (row-tiled packing; also keeps the HAM clock gate fed with
  full-width activity -> 2.4 GHz). One wide exp on ScalarE over the pair's
  [128,1024] psum tile; attn@V per head with V stationary (ones column
  rides as psum row 64 = softmax denominator). Steady state is ACT(exp)-
  bound at ~1.09us/iter with the Scalar engine ~96% busy.
  PSUM: 2x psc[128,1024] ping-pong (4 banks) + 4x ya[65,512] (4 banks).
  ya is 4-deep because a matmul that WARs a recently-read psum buffer
  inherits, via DVE sem-increment coalescing, a wait on everything behind
  that read in the DVE FIFO -- with 4 buffers the WAR lands 2 pairs back.
  Normalize reads ya psum directly (no evacuation copies): denominator
  rows -> partition 0, gpsimd broadcast, reciprocal_approx_fast, two DVE
  multiplies writing bf16 yT. Out-projection runs as a tail reusing the
  psc pool banks (interleaving it into the attention loop needs a free
  psum bank, and all 8 are committed).
"""
import sys
import os
from contextlib import ExitStack

if '/opt/trn_rl_repo' not in sys.path:
    sys.path.insert(0, '/opt/trn_rl_repo')

import numpy as np
import ml_dtypes

bf16 = ml_dtypes.bfloat16

T = 4096
D = 1024
HL = 8          # local heads per core
HD = 64
NT = T // 128   # 32 t-tiles
KT = D // 128   # 8 contraction tiles for projections
NCH = 8         # chunks of 512 along t for attention
CW = 512        # chunk width (query columns per head per iteration)
PAIRS = 4       # head pairs per core
EPS = 1.1920928955078125e-07


def build():
    from concourse import bacc, tile, mybir

    BF16 = mybir.dt.bfloat16
    F32 = mybir.dt.float32
    AF = mybir.ActivationFunctionType
    ALU = mybir.AluOpType
    AX = mybir.AxisListType

    nc = bacc.Bacc()
    # x pre-arranged host-side to [partition, tile*KT*128] so every per-tile
    # load is one contiguous 2KB-per-partition slab (the old transposed
    # gather burned ~90us per DMA queue in descriptors and gated startup)
    xP = nc.declare_dram_parameter("xP", [128, NT * KT * 128], BF16,
                                   isOutput=False)
    wqT = nc.declare_dram_parameter("wqT", [D, 512], BF16, isOutput=False)
    wkT = nc.declare_dram_parameter("wkT", [D, 512], BF16, isOutput=False)
    wvT = nc.declare_dram_parameter("wvT", [D, 512], BF16, isOutput=False)
    woT = nc.declare_dram_parameter("woT", [512, D], BF16, isOutput=False)
    cos2 = nc.declare_dram_parameter("cos2", [128, NT * 64], BF16,
                                     isOutput=False)
    ss = nc.declare_dram_parameter("ss", [128, NT * 32], BF16,
                                   isOutput=False)
    ident = nc.declare_dram_parameter("ident", [128, 128], BF16, isOutput=False)
    out = nc.declare_dram_parameter("out", [T, D], F32, isOutput=True)

    with tile.TileContext(nc) as tc:
        with tc.tile_pool(name="persist", bufs=1) as persist:
            # ---- persistent stores (live across both phases) ----
            # qT split per 1024-chunk so phase 2 chunk c only depends on its
            # own 8 q t-tiles; kT/vaug are needed in full by every chunk.
            qTc = [persist.tile([128, PAIRS, 1024], BF16, tag=f"qT{c}",
                                name=f"qT{c}") for c in range(4)]
            kT = persist.tile([128, PAIRS, T], BF16, tag="kT")
            vaug = persist.tile([128, NT, HL, 65], BF16, tag="vaug")
            wo_sb = persist.tile([128, 4, D], BF16, tag="wo_sb")
            id_sb = persist.tile([128, 128], BF16, tag="id_sb")
            eps_t = persist.tile([128, 1], F32, tag="eps_t")

            nc.vector.memset(vaug[:, :, :, 64:65], 1.0)
            nc.vector.memset(eps_t[:], EPS)

            phase1 = ExitStack()
            with phase1:
                wpool = phase1.enter_context(tc.tile_pool(name="wpool", bufs=1))
                xcolp = phase1.enter_context(tc.tile_pool(name="xcolp", bufs=4))
                scr = phase1.enter_context(tc.tile_pool(name="scr", bufs=3))
                small = phase1.enter_context(tc.tile_pool(name="small", bufs=4))
                ps_qkv = phase1.enter_context(
                    tc.tile_pool(name="ps_qkv", bufs=4, space="PSUM"))
                ps_tr = phase1.enter_context(
                    tc.tile_pool(name="ps_tr", bufs=4, space="PSUM"))

                def load_xcol(t):
                    xc = xcolp.tile([128, KT, 128], BF16, tag="xcol")
                    nc.sync.dma_start(
                        xc[:],
                        xP[:, t * 1024:(t + 1) * 1024].rearrange(
                            "p (k c) -> p k c", k=KT))
                    return xc

                # interleave the first x tiles with the weight loads: the
                # first QKV matmul needs xcol(0)+w_q, and each dma_start
                # costs ~600ns of sequencer issue time, so issue order
                # directly sets how soon the PE can start
                xcols = {0: load_xcol(0)}
                w_sb = {}
                cos_sb = wpool.tile([128, NT, 64], BF16, tag="cos_sb")
                ss_sb = wpool.tile([128, NT, 32], BF16, tag="ss_sb")
                for pf, (name, param) in zip(
                        (1, 2, None),
                        (("q", wqT), ("k", wkT), ("v", wvT))):
                    w_sb[name] = wpool.tile(
                        [128, KT, 512], BF16, tag=f"w{name}",
                        name=f"w_{name}_sb")
                    for ki in range(KT):
                        nc.sync.dma_start(
                            w_sb[name][:, ki, :],
                            param[ki * 128:(ki + 1) * 128, :])
                    if name == "q":
                        # cos/sin feed the first rotary (~16us in) -- issue
                        # before the k/v weights so they arrive in time
                        nc.sync.dma_start(
                            cos_sb[:],
                            cos2[:].rearrange("p (t d) -> p t d", t=NT))
                        nc.sync.dma_start(
                            ss_sb[:], ss[:].rearrange("p (t d) -> p t d",
                                                      t=NT))
                        nc.sync.dma_start(id_sb[:], ident[:])
                    if pf is not None:
                        xcols[pf] = load_xcol(pf)
                nc.sync.dma_start(
                    wo_sb[:], woT[:].rearrange("(k p) n -> p k n", p=128))

                # ============ Phase 1: QKV + rotary + rms + transpose =======
                # The q/k transposes of tile t are deferred until after tile
                # t+1's QKV matmuls are issued: the PE FIFO then always has
                # dense work while the rotary chain (DVE) catches up, instead
                # of stalling on the transposes' qn dependency every tile.
                pending_tr = []
                for t in range(NT):
                    xcol = xcols.pop(t) if t in xcols else load_xcol(t)

                    ps = {}
                    for name in ("q", "k", "v"):
                        ps[name] = ps_qkv.tile([128, 512], F32, tag="pqkv",
                                               name=f"ps_{name}")
                        for ki in range(KT):
                            nc.tensor.matmul(
                                ps[name][:], xcol[:, ki, :],
                                w_sb[name][:, ki, :],
                                start=(ki == 0), stop=(ki == KT - 1))

                    # flush previous tile's transposes (PE busy with QKV now)
                    for qn_, store_, tcol_ in pending_tr:
                        for cb in range(4):
                            tp = ps_tr.tile([128, 128], BF16, tag="tp")
                            nc.tensor.transpose(
                                tp[:], qn_[:, cb * 128:(cb + 1) * 128],
                                id_sb[:])
                            nc.scalar.copy(
                                store_[:, cb, tcol_:tcol_ + 128], tp[:])
                    pending_tr = []

                    # v -> vaug rows 0:64 per head (ScalarE copy, bf16 cast)
                    nc.scalar.copy(
                        vaug[:, t, :, 0:64],
                        ps["v"][:].rearrange("p (h d) -> p h d", h=HL))

                    # rotary + rms for q and k, processed as ONE [128, 1024]
                    # block (q cols 0:512, k cols 512:1024) to halve the DVE
                    # per-op overhead.  rms stats come from PRE-rotary values
                    # (rotation preserves the per-head norm).
                    ctb = cos_sb[:, t, :].unsqueeze(1).unsqueeze(1)\
                        .broadcast_to([128, 2, HL, 64])
                    sb = ss_sb[:, t, :].unsqueeze(1).unsqueeze(1)\
                        .broadcast_to([128, 2, HL, 32])
                    qb = scr.tile([128, 1024], BF16, tag="qb", name="qb_qk")
                    nc.scalar.copy(qb[:, 0:512], ps["q"][:])
                    nc.scalar.copy(qb[:, 512:1024], ps["k"][:])
                    sq = scr.tile([128, 1024], BF16, tag="sq", name="sq_qk")
                    nc.vector.tensor_tensor(sq[:], qb[:], qb[:], op=ALU.mult)
                    ms8 = small.tile([128, 2 * HL], F32, tag="ms8",
                                     name="ms8_qk")
                    nc.vector.tensor_reduce(
                        ms8[:], sq[:].rearrange("p (n h d) -> p (n h) d",
                                                n=2, h=HL),
                        axis=AX.X, op=ALU.add)
                    rms = small.tile([128, 2 * HL], F32, tag="rms",
                                     name="rms_qk")
                    nc.scalar.activation(
                        rms[:], ms8[:], AF.Sqrt, scale=1.0 / HD,
                        bias=eps_t[:])
                    rinv = small.tile([128, 2 * HL], F32, tag="rinv",
                                      name="rinv_qk")
                    nc.vector.reciprocal(rinv[:], rms[:])
                    t1 = scr.tile([128, 1024], BF16, tag="t1", name="t1_qk")
                    nc.vector.tensor_tensor(
                        t1[:].rearrange("p (n h d) -> p n h d", n=2, h=HL),
                        qb[:].rearrange("p (n h d) -> p n h d", n=2, h=HL),
                        ctb, op=ALU.mult)
                    qb4 = qb[:].rearrange("p (n h u d) -> p n h u d",
                                          n=2, h=HL, u=2)
                    r = scr.tile([128, 1024], BF16, tag="r", name="r_qk")
                    r4 = r[:].rearrange("p (n h u d) -> p n h u d",
                                        n=2, h=HL, u=2)
                    t14 = t1[:].rearrange("p (n h u d) -> p n h u d",
                                          n=2, h=HL, u=2)
                    nc.vector.tensor_tensor(
                        r4[:, :, :, 0, :], qb4[:, :, :, 1, :], sb,
                        op=ALU.mult)
                    nc.vector.tensor_tensor(
                        r4[:, :, :, 1, :], qb4[:, :, :, 0, :], sb,
                        op=ALU.mult)
                    nc.vector.tensor_tensor(
                        r4[:, :, :, 0, :], t14[:, :, :, 0, :],
                        r4[:, :, :, 0, :], op=ALU.add)
                    nc.vector.tensor_tensor(
                        r4[:, :, :, 1, :], t14[:, :, :, 1, :],
                        r4[:, :, :, 1, :], op=ALU.subtract)
                    qn = scr.tile([128, 1024], BF16, tag="qn", bufs=3,
                                  name="qn_qk")
                    nc.vector.tensor_tensor(
                        qn[:].rearrange("p (m d) -> p m d", m=2 * HL),
                        r[:].rearrange("p (m d) -> p m d", m=2 * HL),
                        rinv[:].unsqueeze(2).broadcast_to([128, 2 * HL, 64]),
                        op=ALU.mult)
                    pending_tr.append((qn[:, 0:512], qTc[t // 8],
                                       (t % 8) * 128))
                    pending_tr.append((qn[:, 512:1024], kT, t * 128))

                # flush the last tile's transposes
                for qn_, store_, tcol_ in pending_tr:
                    for cb in range(4):
                        tp = ps_tr.tile([128, 128], BF16, tag="tp")
                        nc.tensor.transpose(
                            tp[:], qn_[:, cb * 128:(cb + 1) * 128],
                            id_sb[:])
                        nc.scalar.copy(
                            store_[:, cb, tcol_:tcol_ + 128], tp[:])

            # ============ Phase 2: attention ================================
            with (
                tc.tile_pool(name="p2persist", bufs=1) as p2persist,
                tc.tile_pool(name="zp2", bufs=4) as zp2,
                tc.tile_pool(name="nrm", bufs=2) as nrm,
                tc.tile_pool(name="ostg2", bufs=4) as ostg2,
                tc.tile_pool(name="ps_sc", bufs=2, space="PSUM") as ps_sc,
                tc.tile_pool(name="ps_ya", bufs=4, space="PSUM") as ps_ya,
            ):
                yTn = [p2persist.tile([128, PAIRS, CW], BF16, tag=f"yT{c}",
                                      name=f"yT{c}") for c in range(NCH)]

                # PE warm-up: dense FULL-128-row back-to-back matmuls tied to
                # kT so they run right at phase-2 entry. Re-arms the HAM
                # clock gate before the steady loop (64-row bursts do NOT
                # re-arm it -- learned from the previous kernel's trace).
                wup = ps_sc.tile([128, 2 * CW], F32, tag="psc", name="wup")
                for i in range(16):
                    nc.tensor.matmul(
                        wup[:, 0:512], kT[:, 0, 0:128], kT[:, 0, 0:512],
                        start=(i == 0), stop=(i == 15))

                for ch in range(NCH):
                    qt = qTc[ch // 2]
                    qof = (ch % 2) * 512
                    for pr in range(PAIRS):
                        # bufs=4: each pair's accumulators WAR a buffer last
                        # read TWO pairs back (~70us slack) -- a recent-WAR
                        # here stalls the PE FIFO for the whole normalize
                        # chain (sem-increment coalescing on the DVE stream).
                        ya_e = ps_ya.tile([65, CW], F32, tag="pya",
                                          name="ya_e")
                        ya_o = ps_ya.tile([65, CW], F32, tag="pya",
                                          name="ya_o")
                        for s in range(NT):
                            ssl = slice(s * 128, (s + 1) * 128)
                            psc = ps_sc.tile([128, 2 * CW], F32, tag="psc")
                            # pair's score matmuls: adjacent issue, distinct
                            # PE row-groups -> concurrent execution
                            nc.tensor.matmul(
                                psc[:, 0:CW], kT[0:64, pr, ssl],
                                qt[0:64, pr, qof:qof + CW],
                                start=True, stop=True,
                                tile_position=(0, 0))
                            nc.tensor.matmul(
                                psc[:, CW:2 * CW], kT[64:128, pr, ssl],
                                qt[64:128, pr, qof:qof + CW],
                                start=True, stop=True,
                                tile_position=(64, 0))
                            z = zp2.tile([128, 2 * CW], BF16, tag="z")
                            nc.scalar.activation(
                                z[:], psc[:], AF.Exp, scale=0.125)
                            nc.tensor.matmul(
                                ya_e[:], vaug[:, s, 2 * pr, :], z[:, 0:CW],
                                start=(s == 0), stop=(s == NT - 1))
                            nc.tensor.matmul(
                                ya_o[:], vaug[:, s, 2 * pr + 1, :],
                                z[:, CW:2 * CW],
                                start=(s == 0), stop=(s == NT - 1))
                        # normalize straight out of psum: denominator rows ->
                        # partition 0, broadcast (gpsimd), approx-reciprocal,
                        # then the multiplies read ya PSUM directly and write
                        # normalized bf16 yT (no intermediate evacuation).
                        dtmp = nrm.tile([1, 2 * CW], F32, tag="dtmp", bufs=2)
                        nc.vector.tensor_copy(dtmp[:, 0:CW], ya_e[64:65, :])
                        nc.vector.tensor_copy(dtmp[:, CW:2 * CW],
                                              ya_o[64:65, :])
                        bc = nrm.tile([64, 2 * CW], F32, tag="bc", bufs=2)
                        nc.gpsimd.partition_broadcast(bc[:], dtmp[:],
                                                      channels=64)
                        bcr = nrm.tile([64, 2 * CW], F32, tag="bcr", bufs=2)
                        nc.vector.reciprocal_approx_fast(bcr[:], bc[:])
                        nc.vector.tensor_tensor(
                            yTn[ch][0:64, pr, :], ya_e[0:64, :],
                            bcr[:, 0:CW], op=ALU.mult)
                        nc.vector.tensor_tensor(
                            yTn[ch][64:128, pr, :], ya_o[0:64, :],
                            bcr[:, CW:2 * CW], op=ALU.mult)

                # out-projection tail (PSUM banks freed by the attention
                # loop; po tiles reuse the ps_sc pool)
                for ch in range(NCH):
                    for tt in range(4):
                        po = ps_sc.tile([128, 2 * CW], F32, tag="psc",
                                        name="po")
                        for oc in range(2):
                            for kp in range(4):
                                nc.tensor.matmul(
                                    po[:, oc * 512:(oc + 1) * 512],
                                    yTn[ch][:, kp, tt * 128:(tt + 1) * 128],
                                    wo_sb[:, kp, oc * 512:(oc + 1) * 512],
                                    start=(kp == 0), stop=(kp == 3))
                        ost = ostg2.tile([128, 2 * CW], F32, tag="ost")
                        nc.vector.tensor_copy(ost[:], po[:])
                        tsl = slice(ch * CW + tt * 128,
                                    ch * CW + (tt + 1) * 128)
                        nc.sync.dma_start(out[tsl, :], ost[:])

    nc.compile()
    return nc


_CACHE = {}


def _get_nc():
    if "nc" not in _CACHE:
        _CACHE["nc"] = build()
    return _CACHE["nc"]


def _prep_inputs(x, cos, sin, wq, wk, wv, wo):
    x = np.asarray(x, dtype=np.float32)
    cos = np.asarray(cos, dtype=np.float32).reshape(T, 32)
    sin = np.asarray(sin, dtype=np.float32).reshape(T, 32)
    wq = np.asarray(wq, dtype=np.float32)
    wk = np.asarray(wk, dtype=np.float32)
    wv = np.asarray(wv, dtype=np.float32)
    wo = np.asarray(wo, dtype=np.float32)

    # pre-arrange to the exact SBUF layouts (contiguous per-partition DMAs)
    cos2 = np.ascontiguousarray(
        np.concatenate([cos, cos], axis=1).reshape(32, 128, 64)
        .transpose(1, 0, 2).reshape(128, -1))
    ss = np.ascontiguousarray(
        sin.reshape(32, 128, 32).transpose(1, 0, 2).reshape(128, -1))
    ident = np.eye(128, dtype=bf16)

    in_maps = []
    for c in range(8):
        b, hg = c // 2, c % 2
        rows = slice(hg * 512, (hg + 1) * 512)
        xp = np.ascontiguousarray(
            x[b].T.reshape(8, 128, 32, 128).transpose(1, 2, 0, 3)
            .reshape(128, -1))

        in_maps.append({
            "xP": xp.astype(bf16),
            "wqT": np.ascontiguousarray(wq[rows, :].T).astype(bf16),
            "wkT": np.ascontiguousarray(wk[rows, :].T).astype(bf16),
            "wvT": np.ascontiguousarray(wv[rows, :].T).astype(bf16),
            "woT": np.ascontiguousarray(wo[:, rows].T).astype(bf16),
            "cos2": cos2.astype(bf16),
            "ss": ss.astype(bf16),
            "ident": ident,
        })
    return in_maps


def _run(in_maps, trace=False):
    from concourse.bass_utils import run_bass_kernel_spmd

    nc = _get_nc()
    res = run_bass_kernel_spmd(nc, in_maps, core_ids=list(range(8)),
                               trace=trace)
    parts = [res.results[c]["out"] for c in range(8)]
    full = np.stack([parts[2 * b] + parts[2 * b + 1] for b in range(4)])
    return full.astype(np.float32), res


def kernel(x, cos, sin, wq, wk, wv, wo):
    in_maps = _prep_inputs(x, cos, sin, wq, wk, wv, wo)
    full, _ = _run(in_maps, trace=False)
    return full
